# revision 9
# baseline (speedup 1.0000x reference)
"""AtomAttentionDecoder Trainium2 kernel (8 NeuronCores, SPMD data-parallel).

Sharding: core = b*2 + h  (b in 0..3 batches, h in 0..1 atom-halves of 4096).
Each core:
  - computes a_to_q = a[b] @ W_a2q.T restricted to its atoms' token window
    (windowed: tokens [tok_lo, tok_lo + n_win*128) -- n_win is the max
    window span over all 8 cores so the SPMD graph is shared)
  - gathers a_to_q rows per atom via hardware indirect DMA -> q_new = q + gath
  - one fused matmul per 128-atom tile produces [atom, 35] = [small(32) | Y(3)]
    (W_comb = [W_atom.T | gamma*W_pos.T]); atom_type assembled in SBUF
    (-1e9 fill + strided copy of small + b_atom)
  - LayerNorm via bn_stats/bn_aggr; r_update = rstd*(Y - mu*sWg) + bPos
    (gamma/beta folded into W_comb / sWg / bPos on host)
  - segment-sum via selection matmuls: sel[atom,tok] = (idx == iota), PSUM
    accumulation per 128-token window; res_partial = s_featT @ W_res.T
Host: shards/transposes inputs, sums the two res_type partials per batch,
adds b_res once, reassembles full outputs.
"""

import os
import sys

import numpy as np

for _p in ("/opt/trn_rl_repo",):
    if _p not in sys.path and os.path.isdir(_p):
        sys.path.insert(0, _p)

B, N_ATOM, N_TOK = 4, 8192, 1024
ATOM_S, TFMR_S = 128, 384
LN_EPS = 1e-5
PAD_VAL = -1e9
P = 128
N_CORES = 8
A_SH = N_ATOM // 2          # atoms per core (4096)
N_TILES = A_SH // P         # 32
MASK_SENTINEL = 1 << 20     # idx value for masked atoms (matches no window)

LAST_RESULT = None          # test harness reads exec_time_ns from here


def _build(n_win, sched, first_t, last_t):
    """Build the shared SPMD Bass graph.

    sched: list over tiles t of sorted window lists (union over cores).
    first_t/last_t: window -> first/last tile index hitting it.
    """
    from concourse import bacc, bass, mybir
    import concourse.tile as tile
    from concourse.masks import make_identity

    f32 = mybir.dt.float32
    i32 = mybir.dt.int32
    W = n_win * P
    Alu = mybir.AluOpType

    nc = bacc.Bacc(None, target_bir_lowering=False)

    q_t = nc.dram_tensor("q_t", [P, A_SH], f32, kind="ExternalInput")
    aT = nc.dram_tensor("aT", [P, 3 * W], f32, kind="ExternalInput")
    wa2q = nc.dram_tensor("wa2q", [P, 3 * P], f32, kind="ExternalInput")
    wcomb = nc.dram_tensor("wcomb", [P, 35], f32, kind="ExternalInput")
    wres = nc.dram_tensor("wres", [P, 33], f32, kind="ExternalInput")
    consts = nc.dram_tensor("consts", [P, 38], f32, kind="ExternalInput")
    idx_i = nc.dram_tensor("idx_i", [P, N_TILES], i32, kind="ExternalInput")
    idxm_f = nc.dram_tensor("idxm_f", [P, N_TILES], f32, kind="ExternalInput")

    at_out = nc.dram_tensor("at_out", [A_SH, ATOM_S], f32, kind="ExternalOutput")
    r_out = nc.dram_tensor("r_out", [A_SH, 3], f32, kind="ExternalOutput")
    res_out = nc.dram_tensor("res_out", [W, 33], f32, kind="ExternalOutput")

    a2q_scr = nc.dram_tensor("a2q_scr", [W, ATOM_S], f32)  # internal scratch

    with tile.TileContext(nc) as tc:
        with (
            tc.tile_pool(name="singles", bufs=1) as S,
            tc.tile_pool(name="work", bufs=4) as WK,
            tc.tile_pool(name="selp", bufs=4) as SELP,
            tc.tile_pool(name="stats", bufs=6) as ST,
            tc.tile_pool(name="mm_psum", bufs=4, space="PSUM") as MMP,
            tc.tile_pool(name="sf_psum", bufs=4, space="PSUM") as SFP,
        ):
            # ---- persistent tiles ----
            q_buf = S.tile([P, A_SH], f32)
            aT_sb = S.tile([P, 3 * W], f32)
            wa2q_sb = S.tile([P, 3 * P], f32)
            wcomb_sb = S.tile([P, 35], f32)
            wres_sb = S.tile([P, 33], f32)
            consts_sb = S.tile([P, 38], f32)
            idx_sb = S.tile([P, N_TILES], i32)
            idxm_sb = S.tile([P, N_TILES], f32)
            iota_i = S.tile([P, W], i32)
            iota_f = S.tile([P, W], f32)
            ident = S.tile([P, P], f32)
            a2q_sb = S.tile([P, W], f32)
            sfeat_sb = S.tile([P, W], f32)
            r_sb = S.tile([P, 3 * N_TILES], f32)
            res_sb = S.tile([P, 33 * n_win], f32)
            eps_sb = S.tile([P, 1], f32)
            nc.vector.memset(eps_sb[:], LN_EPS)

            # ---- input DMAs ----
            for c in range(4):
                sl = slice(c * 1024, (c + 1) * 1024)
                nc.sync.dma_start(out=q_buf[:, sl], in_=q_t[:, sl])
            nc.sync.dma_start(out=aT_sb[:], in_=aT[:])
            nc.sync.dma_start(out=wa2q_sb[:], in_=wa2q[:])
            nc.sync.dma_start(out=wcomb_sb[:], in_=wcomb[:])
            nc.sync.dma_start(out=wres_sb[:], in_=wres[:])
            nc.sync.dma_start(out=consts_sb[:], in_=consts[:])
            nc.sync.dma_start(out=idx_sb[:], in_=idx_i[:])
            nc.sync.dma_start(out=idxm_sb[:], in_=idxm_f[:])

            nc.gpsimd.iota(iota_i[:], pattern=[[1, W]], base=0, channel_multiplier=0)
            nc.vector.tensor_copy(out=iota_f[:], in_=iota_i[:])
            make_identity(nc, ident[:])
            nc.gpsimd.memset(sfeat_sb[:], 0.0)

            # ---- phase A: a_to_q for this core's token window ----
            for w in range(n_win):
                ps = MMP.tile([P, P], f32, tag="mm")
                for c in range(3):
                    nc.tensor.matmul(
                        out=ps[:],
                        lhsT=aT_sb[:, c * W + w * P : c * W + (w + 1) * P],
                        rhs=wa2q_sb[:, c * P : (c + 1) * P],
                        start=(c == 0),
                        stop=(c == 2),
                    )
                nc.any.tensor_copy(out=a2q_sb[:, w * P : (w + 1) * P], in_=ps[:])
            nc.sync.dma_start(
                out=a2q_scr[:].rearrange("(w p) d -> p w d", p=P),
                in_=a2q_sb[:].rearrange("p (w d) -> p w d", w=n_win),
            )

            # ---- main loop over 32 atom tiles ----
            sf_tiles = {}
            for t in range(N_TILES):
                tsl = slice(t * P, (t + 1) * P)
                # gather a_to_q rows for this tile's atoms
                gath = WK.tile([P, P], f32, tag="gath")
                nc.gpsimd.indirect_dma_start(
                    out=gath[:],
                    out_offset=None,
                    in_=a2q_scr[:],
                    in_offset=bass.IndirectOffsetOnAxis(ap=idx_sb[:, t : t + 1], axis=0),
                )
                qn = WK.tile([P, P], f32, tag="qn")
                nc.vector.tensor_tensor(
                    out=qn[:], in0=q_buf[:, tsl], in1=gath[:], op=Alu.add
                )

                # transpose q_new for the head matmul
                trp = MMP.tile([P, P], f32, tag="mm")
                nc.tensor.transpose(out=trp[:], in_=qn[:], identity=ident[:])
                qnT = WK.tile([P, P], f32, tag="qnT")
                nc.any.tensor_copy(out=qnT[:], in_=trp[:])

                # fused head matmul: [atom, 35] = q_new @ [W_atom.T | Wg]
                hps = MMP.tile([P, 35], f32, tag="mm")
                nc.tensor.matmul(
                    out=hps[:], lhsT=qnT[:], rhs=wcomb_sb[:], start=True, stop=True
                )

                # layer-norm stats
                stt = ST.tile([P, 6], f32, tag="stt")
                mv = ST.tile([P, 2], f32, tag="mv")
                rstd = ST.tile([P, 1], f32, tag="rstd")
                nc.vector.bn_stats(out=stt[:], in_=qn[:])
                nc.vector.bn_aggr(out=mv[:], in_=stt[:])
                nc.scalar.activation(
                    out=rstd[:],
                    in_=mv[:, 1:2],
                    func=mybir.ActivationFunctionType.Sqrt,
                    bias=eps_sb[:],
                    scale=1.0,
                )
                nc.vector.reciprocal(out=rstd[:], in_=rstd[:])

                # atom_type tile: -1e9 fill, small + b_atom into cols 0,4,...,124
                at = WK.tile([P, ATOM_S], f32, tag="at")
                nc.gpsimd.memset(at[:], PAD_VAL)
                at4 = at[:].rearrange("p (k f) -> p k f", f=4)
                hps3 = hps[:].rearrange("p (k f) -> p k f", f=1)
                cb3 = consts_sb[:].rearrange("p (k f) -> p k f", f=1)
                nc.vector.tensor_tensor(
                    out=at4[:, :, 0:1],
                    in0=hps3[:, 0:32, :],
                    in1=cb3[:, 0:32, :],
                    op=Alu.add,
                )
                nc.sync.dma_start(out=at_out[tsl, :], in_=at[:])

                # r_update epilogue: r = rstd*(Y - mu*sWg) + bPos
                t1 = ST.tile([P, 3], f32, tag="t1")
                t2 = ST.tile([P, 3], f32, tag="t2")
                t3 = ST.tile([P, 3], f32, tag="t3")
                nc.vector.tensor_scalar(
                    out=t1[:],
                    in0=consts_sb[:, 32:35],
                    scalar1=mv[:, 0:1],
                    scalar2=None,
                    op0=Alu.mult,
                )
                nc.vector.tensor_tensor(
                    out=t2[:], in0=hps[:, 32:35], in1=t1[:], op=Alu.subtract
                )
                nc.vector.tensor_scalar(
                    out=t3[:], in0=t2[:], scalar1=rstd[:], scalar2=None, op0=Alu.mult
                )
                nc.vector.tensor_tensor(
                    out=r_sb[:, t * 3 : (t + 1) * 3],
                    in0=t3[:],
                    in1=consts_sb[:, 35:38],
                    op=Alu.add,
                )

                # segment-sum: selection matmuls into per-window PSUM accumulators
                for w in sched[t]:
                    sel = SELP.tile([P, P], f32, tag="sel")
                    nc.vector.tensor_tensor(
                        out=sel[:],
                        in0=idxm_sb[:, t : t + 1].to_broadcast([P, P]),
                        in1=iota_f[:, w * P : (w + 1) * P],
                        op=Alu.is_equal,
                    )
                    if w not in sf_tiles:
                        sf_tiles[w] = SFP.tile([P, P], f32, tag="sf", name=f"sf{w}")
                    nc.tensor.matmul(
                        out=sf_tiles[w][:],
                        lhsT=qn[:],
                        rhs=sel[:],
                        start=(t == first_t[w]),
                        stop=(t == last_t[w]),
                        skip_group_check=True,
                    )
                    if t == last_t[w]:
                        nc.any.tensor_copy(
                            out=sfeat_sb[:, w * P : (w + 1) * P], in_=sf_tiles[w][:]
                        )
                        del sf_tiles[w]

            # ---- res head: res_partial = s_featT.T @ W_res.T ----
            for w in range(n_win):
                rps = MMP.tile([P, 33], f32, tag="mm")
                nc.tensor.matmul(
                    out=rps[:],
                    lhsT=sfeat_sb[:, w * P : (w + 1) * P],
                    rhs=wres_sb[:],
                    start=True,
                    stop=True,
                )
                nc.any.tensor_copy(out=res_sb[:, w * 33 : (w + 1) * 33], in_=rps[:])

            nc.sync.dma_start(
                out=res_out[:].rearrange("(w p) k -> p w k", p=P),
                in_=res_sb[:].rearrange("p (w k) -> p w k", w=n_win),
            )
            nc.sync.dma_start(
                out=r_out[:].rearrange("(t p) o -> p t o", p=P),
                in_=r_sb[:].rearrange("p (t o) -> p t o", t=N_TILES),
            )

    nc.compile()
    return nc


def kernel(
    a,
    q,
    c,
    atom_tok_idx,
    atom_to_token,
    atom_pad_mask,
    W_a2q,
    ln_gamma,
    ln_beta,
    W_pos,
    W_res,
    b_res,
    W_atom,
    b_atom,
    allowed_idx,
    **_unused,
):
    global LAST_RESULT
    from concourse.bass_utils import run_bass_kernel_spmd

    a = np.asarray(a, np.float32)
    q = np.asarray(q, np.float32)
    idx_all = np.asarray(atom_tok_idx).astype(np.int64)
    mask_all = np.asarray(atom_pad_mask).astype(bool)
    W_a2q = np.asarray(W_a2q, np.float32)
    ln_gamma = np.asarray(ln_gamma, np.float32)
    ln_beta = np.asarray(ln_beta, np.float32)
    W_pos = np.asarray(W_pos, np.float32)
    W_res = np.asarray(W_res, np.float32)
    b_res = np.asarray(b_res, np.float32)
    W_atom = np.asarray(W_atom, np.float32)
    b_atom = np.asarray(b_atom, np.float32)
    allowed = np.asarray(allowed_idx).astype(np.int64)

    # ---- per-core shard metadata ----
    cores = []
    for core in range(N_CORES):
        b, h = divmod(core, 2)
        sl = slice(h * A_SH, (h + 1) * A_SH)
        idx = idx_all[b, sl]
        mask = mask_all[b, sl]
        tok_lo = int(idx.min()) // P * P
        span = (int(idx.max()) + P) // P - tok_lo // P
        cores.append({"b": b, "sl": sl, "idx": idx, "mask": mask,
                      "tok_lo": tok_lo, "span": span})
    n_win = max(ci["span"] for ci in cores)
    W = n_win * P

    # union window schedule over cores (shared SPMD graph)
    sched_sets = [set() for _ in range(N_TILES)]
    for ci in cores:
        idx_reb = ci["idx"] - ci["tok_lo"]
        idxm = np.where(ci["mask"], idx_reb, MASK_SENTINEL)
        ci["idx_reb"] = idx_reb
        ci["idxm"] = idxm
        for t in range(N_TILES):
            ws = np.unique(idxm[t * P : (t + 1) * P] // P)
            for w in ws:
                if 0 <= w < n_win:
                    sched_sets[t].add(int(w))
    sched = [sorted(s) for s in sched_sets]
    first_t, last_t = {}, {}
    for t in range(N_TILES):
        for w in sched[t]:
            first_t.setdefault(w, t)
            last_t[w] = t

    # ---- shared host-folded weights ----
    Wg = ln_gamma[:, None] * W_pos.T                      # [128, 3]
    sWg = Wg.sum(axis=0)                                  # [3]
    bPos = ln_beta @ W_pos.T                              # [3]
    wcomb_np = np.hstack([W_atom.T, Wg]).astype(np.float32)          # [128, 35]
    wres_np = np.ascontiguousarray(W_res.T, np.float32)              # [128, 33]
    wa2q_np = (
        W_a2q.T.reshape(3, P, P).transpose(1, 0, 2).reshape(P, 3 * P).copy()
    )
    consts_np = np.broadcast_to(
        np.concatenate([b_atom, sWg, bPos]).astype(np.float32)[None, :], (P, 38)
    ).copy()

    # ---- per-core input maps ----
    in_maps = []
    for ci in cores:
        b, sl, tok_lo = ci["b"], ci["sl"], ci["tok_lo"]
        aT_w = np.zeros((TFMR_S, W), np.float32)
        hi = min(tok_lo + W, N_TOK)
        aT_w[:, : hi - tok_lo] = a[b].T[:, tok_lo:hi]
        aT_np = aT_w.reshape(3, P, W).transpose(1, 0, 2).reshape(P, 3 * W).copy()
        q_np = (
            q[b, sl].reshape(N_TILES, P, P).transpose(1, 0, 2).reshape(P, A_SH).copy()
        )
        idx_np = (
            ci["idx_reb"].astype(np.int32).reshape(N_TILES, P).T.copy()
        )
        idxm_np = ci["idxm"].astype(np.float32).reshape(N_TILES, P).T.copy()
        in_maps.append(
            {
                "q_t": q_np,
                "aT": aT_np,
                "wa2q": wa2q_np,
                "wcomb": wcomb_np,
                "wres": wres_np,
                "consts": consts_np,
                "idx_i": idx_np,
                "idxm_f": idxm_np,
            }
        )

    nc = _build(n_win, sched, first_t, last_t)
    LAST_RESULT = run_bass_kernel_spmd(nc, in_maps, core_ids=list(range(N_CORES)))
    results = LAST_RESULT.results

    # ---- host assembly ----
    r_update = np.empty((B, N_ATOM, 3), np.float32)
    atom_type = np.empty((B, N_ATOM, 128), np.float32)
    res_type = np.broadcast_to(b_res.astype(np.float32), (B, N_TOK, 33)).copy()
    for core, ci in enumerate(cores):
        b, sl, tok_lo = ci["b"], ci["sl"], ci["tok_lo"]
        r_update[b, sl] = results[core]["r_out"]
        atom_type[b, sl] = results[core]["at_out"]
        hi = min(tok_lo + W, N_TOK)
        res_type[b, tok_lo:hi] += results[core]["res_out"][: hi - tok_lo]
    return (r_update, res_type, atom_type)


# revision 12
# speedup vs baseline: 1.5011x; 1.5011x over previous
"""AtomAttentionDecoder Trainium2 kernel (8 NeuronCores, SPMD data-parallel).

Sharding: core = b*2 + h. Batch b owns its atoms; half h owns the atoms whose
(sorted) token index falls in [h*512, (h+1)*512) -- variable count, padded to
a common A_PAD. Token-boundary sharding keeps the per-tile token->window maps
nearly identical across cores, so the shared SPMD schedule stays tight, and
res_type halves are disjoint (no cross-core reduction at all).

Per core (all matmul operands bf16, PSUM f32):
  phase A:  a2q_w = a @ W_a2q.T and ha_w = a @ (W_a2q.T @ Wcomb) for the
            core's 512 tokens; rows packed as gh = [a2q | ha + b_atom] in SBUF
  gather:   selT[tok,atom] = (idx[atom] == tok) built by DVE/GpSimd from a
            partition-broadcast idx row and a per-window iota column;
            PE matmul gath[atom, 0:163] = selT.T @ gh  (accumulated over the
            1-2 windows a 128-atom tile can touch)
  qn = q + gath[:, :128];  head hps = qn @ [W_atom.T | gamma*W_pos.T] via
            host-transposed qT (lhsT) + gath[:, 128:163]
  scatter:  sel[atom,tok] = (idxm == iota); s_featT += qn.T @ sel per window
  LN:       bn_stats/bn_aggr per tile into column buffers; rstd/r_update
            computed afterwards in a handful of batched [128, NT*3] ops
            (r = rstd*(Y - mu*sWg) + bPos with gamma/beta folded on host)
  res_partial = s_featT.T @ W_res.T   (exact, disjoint tokens per core)
Host: shard bookkeeping, bf16 casts/transposes, final assembly + b_res.
"""

import os
import sys

import numpy as np

for _p in ("/opt/trn_rl_repo",):
    if _p not in sys.path and os.path.isdir(_p):
        sys.path.insert(0, _p)

import ml_dtypes

B, N_ATOM, N_TOK = 4, 8192, 1024
ATOM_S, TFMR_S = 128, 384
LN_EPS = 1e-5
PAD_VAL = -1e9
P = 128
N_CORES = 8
TOK_SH = N_TOK // 2         # tokens per core (512)
N_WIN = TOK_SH // P         # 4 windows of 128 tokens
GH_W = ATOM_S + 35          # gather row: [a2q(128) | ha(35)]
MASK_SENTINEL = 100000.0

LAST_RESULT = None


def _build(NT, gsched, ssched, sfirst, slast):
    from concourse import bacc, bass, mybir
    import concourse.tile as tile

    f32 = mybir.dt.float32
    bf16 = mybir.dt.bfloat16
    i32 = mybir.dt.int32
    Alu = mybir.AluOpType
    A_PAD = NT * P

    nc = bacc.Bacc(None, target_bir_lowering=False)

    q_t = nc.dram_tensor("q_t", [P, A_PAD], bf16, kind="ExternalInput")
    qT = nc.dram_tensor("qT", [P, A_PAD], bf16, kind="ExternalInput")
    aT = nc.dram_tensor("aT", [P, 3 * TOK_SH], bf16, kind="ExternalInput")
    wa2q = nc.dram_tensor("wa2q", [P, 3 * P], bf16, kind="ExternalInput")
    wcomb = nc.dram_tensor("wcomb", [P, 35], bf16, kind="ExternalInput")
    wcomb2 = nc.dram_tensor("wcomb2", [P, 3 * 35], bf16, kind="ExternalInput")
    wres = nc.dram_tensor("wres", [P, 33], bf16, kind="ExternalInput")
    consts = nc.dram_tensor("consts", [P, 41], f32, kind="ExternalInput")
    idxg = nc.dram_tensor("idxg", [1, A_PAD], f32, kind="ExternalInput")
    idxm_f = nc.dram_tensor("idxm_f", [P, NT], f32, kind="ExternalInput")

    at_out = nc.dram_tensor("at_out", [A_PAD, ATOM_S], f32, kind="ExternalOutput")
    r_out = nc.dram_tensor("r_out", [A_PAD, 3], f32, kind="ExternalOutput")
    res_out = nc.dram_tensor("res_out", [TOK_SH, 33], f32, kind="ExternalOutput")

    with tile.TileContext(nc) as tc:
        with (
            tc.tile_pool(name="singles", bufs=1) as S,
            tc.tile_pool(name="work", bufs=4) as WK,
            tc.tile_pool(name="selp", bufs=6) as SELP,
            tc.tile_pool(name="stats", bufs=4) as ST,
            tc.tile_pool(name="mm_psum", bufs=4, space="PSUM") as MMP,
            tc.tile_pool(name="sf_psum", bufs=3, space="PSUM") as SFP,
        ):
            # ---- persistent tiles ----
            q_buf = S.tile([P, A_PAD], bf16)
            qT_buf = S.tile([P, A_PAD], bf16)
            aT_sb = S.tile([P, 3 * TOK_SH], bf16)
            wa2q_sb = S.tile([P, 3 * P], bf16)
            wcomb_sb = S.tile([P, 35], bf16)
            wcomb2_sb = S.tile([P, 3 * 35], bf16)
            wres_sb = S.tile([P, 33], bf16)
            consts_sb = S.tile([P, 41], f32)
            idxm_sb = S.tile([P, NT], f32)
            idxT_sb = S.tile([P, A_PAD], f32)
            iota_pi = S.tile([P, N_WIN], i32)
            iota_pf = S.tile([P, N_WIN], f32)
            iotaT_i = S.tile([P, TOK_SH], i32)
            iotaT_f = S.tile([P, TOK_SH], f32)
            gh_sb = S.tile([P, N_WIN * GH_W], bf16)
            sfeatT_sb = S.tile([P, TOK_SH], bf16)
            Y_all = S.tile([P, 3 * NT], f32)
            stats_all = S.tile([P, 2 * NT], f32)
            rstd_sb = S.tile([P, NT], f32)
            z_sb = S.tile([P, NT], f32)
            rY_sb = S.tile([P, 3 * NT], f32)
            r_sb = S.tile([P, 3 * NT], f32)
            res_sb = S.tile([P, N_WIN * 33], f32)
            eps_sb = S.tile([P, 1], f32)

            # ---- loads & constants ----
            for c in range(4):
                sl = slice(c * (A_PAD // 4), (c + 1) * (A_PAD // 4))
                nc.sync.dma_start(out=q_buf[:, sl], in_=q_t[:, sl])
                nc.sync.dma_start(out=qT_buf[:, sl], in_=qT[:, sl])
            nc.sync.dma_start(out=aT_sb[:], in_=aT[:])
            nc.sync.dma_start(out=wa2q_sb[:], in_=wa2q[:])
            nc.sync.dma_start(out=wcomb_sb[:], in_=wcomb[:])
            nc.sync.dma_start(out=wcomb2_sb[:], in_=wcomb2[:])
            nc.sync.dma_start(out=wres_sb[:], in_=wres[:])
            nc.sync.dma_start(out=consts_sb[:], in_=consts[:])
            nc.sync.dma_start(out=idxm_sb[:], in_=idxm_f[:])
            # partition-broadcast of the idx row: every partition gets idx[a]
            idxg_bcast = bass.AP(
                tensor=idxg[:].tensor, offset=0, ap=[[0, P], [1, A_PAD]]
            )
            nc.sync.dma_start(out=idxT_sb[:], in_=idxg_bcast)

            nc.vector.memset(eps_sb[:], LN_EPS)
            nc.gpsimd.iota(iota_pi[:], pattern=[[P, N_WIN]], base=0, channel_multiplier=1)
            nc.vector.tensor_copy(out=iota_pf[:], in_=iota_pi[:])
            nc.gpsimd.iota(iotaT_i[:], pattern=[[1, TOK_SH]], base=0, channel_multiplier=0)
            nc.vector.tensor_copy(out=iotaT_f[:], in_=iotaT_i[:])
            nc.gpsimd.memset(sfeatT_sb[:], 0.0)

            # ---- phase A: gh = [a2q | ha + b_atom] for this core's tokens ----
            for w in range(N_WIN):
                aps = MMP.tile([P, P], f32, tag="mm", name=f"aps{w}")
                for c in range(3):
                    nc.tensor.matmul(
                        out=aps[:],
                        lhsT=aT_sb[:, c * TOK_SH + w * P : c * TOK_SH + (w + 1) * P],
                        rhs=wa2q_sb[:, c * P : (c + 1) * P],
                        start=(c == 0),
                        stop=(c == 2),
                    )
                nc.any.tensor_copy(
                    out=gh_sb[:, w * GH_W : w * GH_W + P], in_=aps[:]
                )
                hps = MMP.tile([P, 35], f32, tag="mm", name=f"haps{w}")
                for c in range(3):
                    nc.tensor.matmul(
                        out=hps[:],
                        lhsT=aT_sb[:, c * TOK_SH + w * P : c * TOK_SH + (w + 1) * P],
                        rhs=wcomb2_sb[:, c * 35 : (c + 1) * 35],
                        start=(c == 0),
                        stop=(c == 2),
                    )
                nc.vector.tensor_tensor(
                    out=gh_sb[:, w * GH_W + P : (w + 1) * GH_W],
                    in0=hps[:],
                    in1=consts_sb[:, 0:35],
                    op=Alu.add,
                )

            # ---- main loop over atom tiles ----
            sf_tiles = {}
            for t in range(NT):
                tsl = slice(t * P, (t + 1) * P)
                # gather via selection matmuls: gath[atom, 0:163]
                gps = MMP.tile([P, GH_W], f32, tag="mm", name=f"gps{t}")
                gws = gsched[t]
                for j, w in enumerate(gws):
                    selT = SELP.tile([P, P], bf16, tag="selT", name=f"selT{t}_{w}")
                    eng = nc.vector
                    eng.tensor_tensor(
                        out=selT[:],
                        in0=idxT_sb[:, tsl],
                        in1=iota_pf[:, w : w + 1].to_broadcast([P, P]),
                        op=Alu.is_equal,
                    )
                    nc.tensor.matmul(
                        out=gps[:],
                        lhsT=selT[:],
                        rhs=gh_sb[:, w * GH_W : (w + 1) * GH_W],
                        start=(j == 0),
                        stop=(j == len(gws) - 1),
                    )
                # qn = q + gath (bf16 result)
                qn = WK.tile([P, P], bf16, tag="qn", name=f"qn{t}")
                nc.vector.tensor_tensor(
                    out=qn[:], in0=q_buf[:, tsl], in1=gps[:, 0:P], op=Alu.add
                )
                # stage the ha part of the gather to SBUF (single-psum-operand rule)
                gsb = WK.tile([P, 35], f32, tag="gsb", name=f"gsb{t}")
                nc.any.tensor_copy(out=gsb[:], in_=gps[:, P:GH_W])

                # head: q @ [W_atom.T | Wg]
                hps = MMP.tile([P, 35], f32, tag="mm", name=f"hps{t}")
                nc.tensor.matmul(
                    out=hps[:], lhsT=qT_buf[:, tsl], rhs=wcomb_sb[:],
                    start=True, stop=True,
                )

                # LN stats into column buffers
                stt = ST.tile([P, 6], f32, tag="stt", name=f"stt{t}")
                nc.vector.bn_stats(out=stt[:], in_=qn[:])
                nc.vector.bn_aggr(out=stats_all[:, 2 * t : 2 * t + 2], in_=stt[:])

                # atom_type tile
                at = WK.tile([P, ATOM_S], f32, tag="at", name=f"at{t}")
                nc.any.memset(at[:], PAD_VAL)
                at4 = at[:].rearrange("p (k f) -> p k f", f=4)
                hps3 = hps[:].rearrange("p (k f) -> p k f", f=1)
                gsb3 = gsb[:].rearrange("p (k f) -> p k f", f=1)
                nc.vector.tensor_tensor(
                    out=at4[:, :, 0:1],
                    in0=hps3[:, 0:32, :],
                    in1=gsb3[:, 0:32, :],
                    op=Alu.add,
                )
                nc.sync.dma_start(out=at_out[tsl, :], in_=at[:])

                # Y staging for the r_update epilogue
                nc.vector.tensor_tensor(
                    out=Y_all[:, 3 * t : 3 * t + 3],
                    in0=hps[:, 32:35],
                    in1=gsb[:, 32:35],
                    op=Alu.add,
                )

                # scatter (segment-sum) into per-window PSUM accumulators
                for j, w in enumerate(ssched[t]):
                    sel = SELP.tile([P, P], bf16, tag="sel", name=f"sel{t}_{w}")
                    eng = nc.vector
                    eng.tensor_tensor(
                        out=sel[:],
                        in0=idxm_sb[:, t : t + 1].to_broadcast([P, P]),
                        in1=iotaT_f[:, w * P : (w + 1) * P],
                        op=Alu.is_equal,
                    )
                    if w not in sf_tiles:
                        sf_tiles[w] = SFP.tile([P, P], f32, tag="sf", name=f"sf{w}")
                    nc.tensor.matmul(
                        out=sf_tiles[w][:],
                        lhsT=qn[:],
                        rhs=sel[:],
                        start=(t == sfirst[w]),
                        stop=(t == slast[w]),
                        skip_group_check=True,
                    )
                    if t == slast[w]:
                        nc.any.tensor_copy(
                            out=sfeatT_sb[:, w * P : (w + 1) * P], in_=sf_tiles[w][:]
                        )
                        del sf_tiles[w]

            # ---- batched r_update epilogue ----
            st3 = stats_all[:].rearrange("p (t two) -> p t two", two=2)
            mu3 = st3[:, :, 0:1]
            var3 = st3[:, :, 1:2]
            rstd3 = rstd_sb[:].rearrange("p (t one) -> p t one", one=1)
            nc.scalar.activation(
                out=rstd3,
                in_=var3,
                func=mybir.ActivationFunctionType.Sqrt,
                bias=eps_sb[:],
                scale=1.0,
            )
            nc.vector.reciprocal(out=rstd_sb[:], in_=rstd_sb[:])
            z3 = z_sb[:].rearrange("p (t one) -> p t one", one=1)
            nc.vector.tensor_tensor(out=z3, in0=mu3, in1=rstd3, op=Alu.mult)
            Y3 = Y_all[:].rearrange("p (t o) -> p t o", o=3)
            rY3 = rY_sb[:].rearrange("p (t o) -> p t o", o=3)
            nc.vector.tensor_tensor(
                out=rY3, in0=Y3, in1=rstd3.to_broadcast([P, NT, 3]), op=Alu.mult
            )
            negsWg3 = (
                consts_sb[:, 35:38]
                .rearrange("p (one o) -> p one o", one=1)
                .to_broadcast([P, NT, 3])
            )
            bpos3 = (
                consts_sb[:, 38:41]
                .rearrange("p (one o) -> p one o", one=1)
                .to_broadcast([P, NT, 3])
            )
            r3 = r_sb[:].rearrange("p (t o) -> p t o", o=3)
            nc.vector.tensor_tensor(
                out=r3, in0=z3.to_broadcast([P, NT, 3]), in1=negsWg3, op=Alu.mult
            )
            nc.vector.tensor_tensor(out=r3, in0=r3, in1=rY3, op=Alu.add)
            nc.vector.tensor_tensor(out=r3, in0=r3, in1=bpos3, op=Alu.add)
            nc.sync.dma_start(
                out=r_out[:].rearrange("(t p) o -> p t o", p=P), in_=r3
            )

            # ---- res head ----
            for w in range(N_WIN):
                rps = MMP.tile([P, 33], f32, tag="mm", name=f"rps{w}")
                nc.tensor.matmul(
                    out=rps[:],
                    lhsT=sfeatT_sb[:, w * P : (w + 1) * P],
                    rhs=wres_sb[:],
                    start=True,
                    stop=True,
                )
                nc.any.tensor_copy(out=res_sb[:, w * 33 : (w + 1) * 33], in_=rps[:])
            nc.sync.dma_start(
                out=res_out[:].rearrange("(w p) k -> p w k", p=P),
                in_=res_sb[:].rearrange("p (w k) -> p w k", w=N_WIN),
            )

    nc.compile()
    return nc


def kernel(
    a,
    q,
    c,
    atom_tok_idx,
    atom_to_token,
    atom_pad_mask,
    W_a2q,
    ln_gamma,
    ln_beta,
    W_pos,
    W_res,
    b_res,
    W_atom,
    b_atom,
    allowed_idx,
    **_unused,
):
    global LAST_RESULT
    from concourse.bass_utils import run_bass_kernel_spmd

    bf = ml_dtypes.bfloat16
    a = np.asarray(a, np.float32)
    q = np.asarray(q, np.float32)
    idx_all = np.asarray(atom_tok_idx).astype(np.int64)
    mask_all = np.asarray(atom_pad_mask).astype(bool)
    W_a2q = np.asarray(W_a2q, np.float32)
    ln_gamma = np.asarray(ln_gamma, np.float32)
    ln_beta = np.asarray(ln_beta, np.float32)
    W_pos = np.asarray(W_pos, np.float32)
    W_res = np.asarray(W_res, np.float32)
    b_res = np.asarray(b_res, np.float32)
    W_atom = np.asarray(W_atom, np.float32)
    b_atom = np.asarray(b_atom, np.float32)

    # ---- shard boundaries (token-sharded halves) ----
    cores = []
    for core in range(N_CORES):
        b, h = divmod(core, 2)
        cut = int(np.searchsorted(idx_all[b], TOK_SH))
        lo, hi = (0, cut) if h == 0 else (cut, N_ATOM)
        cores.append({"b": b, "h": h, "lo": lo, "hi": hi, "cnt": hi - lo})
    max_cnt = max(ci["cnt"] for ci in cores)
    NT = (max_cnt + P - 1) // P
    A_PAD = NT * P

    # ---- per-core indices and schedules ----
    gsched_sets = [set() for _ in range(NT)]
    ssched_sets = [set() for _ in range(NT)]
    for ci in cores:
        b, h, lo, hi, cnt = ci["b"], ci["h"], ci["lo"], ci["hi"], ci["cnt"]
        idx_reb = (idx_all[b, lo:hi] - h * TOK_SH).astype(np.float32)
        mask = mask_all[b, lo:hi]
        idxg = np.full(A_PAD, MASK_SENTINEL, np.float32)
        idxg[:cnt] = idx_reb
        idxm = np.full(A_PAD, MASK_SENTINEL, np.float32)
        idxm[:cnt] = np.where(mask, idx_reb, MASK_SENTINEL)
        ci["idxg"] = idxg
        ci["idxm"] = idxm
        for t in range(NT):
            seg_g = idxg[t * P : (t + 1) * P]
            seg_s = idxm[t * P : (t + 1) * P]
            for w in np.unique(seg_g // P):
                if 0 <= w < N_WIN:
                    gsched_sets[t].add(int(w))
            for w in np.unique(seg_s // P):
                if 0 <= w < N_WIN:
                    ssched_sets[t].add(int(w))
    gsched = [sorted(s) for s in gsched_sets]
    ssched = [sorted(s) for s in ssched_sets]
    sfirst, slast = {}, {}
    for t in range(NT):
        for w in ssched[t]:
            sfirst.setdefault(w, t)
            slast[w] = t

    # ---- host-folded weights ----
    Wg = ln_gamma[:, None] * W_pos.T                       # [128, 3]
    sWg = Wg.sum(axis=0)
    bpos = ln_beta @ W_pos.T
    wcomb_np = np.hstack([W_atom.T, Wg]).astype(bf)        # [128, 35]
    wcomb2_f = W_a2q.T @ np.hstack([W_atom.T, Wg])         # [384, 35]
    wcomb2_np = (
        wcomb2_f.reshape(3, P, 35).transpose(1, 0, 2).reshape(P, 3 * 35).astype(bf)
    )
    wa2q_np = (
        W_a2q.T.reshape(3, P, P).transpose(1, 0, 2).reshape(P, 3 * P).astype(bf)
    )
    wres_np = np.ascontiguousarray(W_res.T).astype(bf)     # [128, 33]
    consts_np = np.broadcast_to(
        np.concatenate(
            [b_atom, np.zeros(3, np.float32), -sWg, bpos]
        ).astype(np.float32)[None, :],
        (P, 41),
    ).copy()

    # ---- per-core input maps ----
    in_maps = []
    for ci in cores:
        b, h, lo, hi, cnt = ci["b"], ci["h"], ci["lo"], ci["hi"], ci["cnt"]
        q_sh = np.zeros((A_PAD, ATOM_S), np.float32)
        q_sh[:cnt] = q[b, lo:hi]
        q_t_np = (
            q_sh.reshape(NT, P, ATOM_S).transpose(1, 0, 2).reshape(P, A_PAD).astype(bf)
        )
        qT_np = np.ascontiguousarray(q_sh.T).astype(bf)    # [128, A_PAD]
        aT_np = (
            a[b].T[:, h * TOK_SH : (h + 1) * TOK_SH]
            .reshape(3, P, TOK_SH)
            .transpose(1, 0, 2)
            .reshape(P, 3 * TOK_SH)
            .astype(bf)
        )
        idxm_np = ci["idxm"].reshape(NT, P).T.copy()       # [128, NT]
        in_maps.append(
            {
                "q_t": q_t_np,
                "qT": qT_np,
                "aT": aT_np,
                "wa2q": wa2q_np,
                "wcomb": wcomb_np,
                "wcomb2": wcomb2_np,
                "wres": wres_np,
                "consts": consts_np,
                "idxg": ci["idxg"][None, :],
                "idxm_f": idxm_np,
            }
        )

    nc = _build(NT, gsched, ssched, sfirst, slast)
    LAST_RESULT = run_bass_kernel_spmd(nc, in_maps, core_ids=list(range(N_CORES)))
    results = LAST_RESULT.results

    # ---- host assembly ----
    r_update = np.empty((B, N_ATOM, 3), np.float32)
    atom_type = np.empty((B, N_ATOM, ATOM_S), np.float32)
    res_type = np.broadcast_to(b_res.astype(np.float32), (B, N_TOK, 33)).copy()
    for core, ci in enumerate(cores):
        b, h, lo, hi, cnt = ci["b"], ci["h"], ci["lo"], ci["hi"], ci["cnt"]
        r_update[b, lo:hi] = results[core]["r_out"][:cnt]
        atom_type[b, lo:hi] = results[core]["at_out"][:cnt]
        res_type[b, h * TOK_SH : (h + 1) * TOK_SH] += results[core]["res_out"]
    return (r_update, res_type, atom_type)


# revision 20
# speedup vs baseline: 1.7576x; 1.1709x over previous
"""AtomAttentionDecoder Trainium2 kernel (8 NeuronCores, SPMD data-parallel).

Sharding: core = b*2 + h. Batch b owns its atoms; half h owns the atoms whose
(sorted) token index falls in [h*512, (h+1)*512) -- variable count, padded to
a common A_PAD. Token-boundary sharding keeps the per-tile token->window maps
nearly identical across cores (tight shared SPMD schedule) and makes the
res_type halves disjoint (no cross-core reduction).

Per core (all matmul operands f16, PSUM f32):
  phase A:  one fused matmul set produces gh = [a2q(128) | a2q@W_res.T(33) |
            a2q@[W_atom.T|Wg](35) + b_atom] for the core's 512 tokens, where
            a2q = a @ W_a2q.T  (ha parts via host-folded W_a2q.T @ W).
  gather:   selT[tok,atom] = (idx[atom] == tok) from a partition-broadcast
            idx row (f16) vs per-window iota column; PE matmul accumulates
            gps[atom, 0:196] = selT.T @ gh over the tile's 1-2 windows; the
            head matmul (lhsT = host-transposed qT, rhs = [W_res.T|W_atom.T|Wg])
            then accumulates q's own contribution onto gps[:, 128:196].
  qn' = [q|0] + gps[:, 0:161]  ->  qn (LN stats input), qr = qn @ W_res.T
  scatter:  sel[atom,tok] = (idxm == iota); res_partialT[33, tok] += qr.T @ sel
            -- a single [33, 512] PSUM bank accumulates the whole segment-sum
            head; atom_type/Y come from gps[:, 161:196] via ACT copies.
  LN:       bn_stats per tile into a column buffer (count/mean/count*var
            triplets -- no bn_aggr); rstd + r_update = rstd*(Y - mu*sWg) + bPos
            computed in a few batched [128, NT*3] ops with stride-0 views.
Host: shard bookkeeping, f16 casts/transposes, final assembly + b_res.
"""

import os
import sys

import numpy as np

for _p in ("/opt/trn_rl_repo",):
    if _p not in sys.path and os.path.isdir(_p):
        sys.path.insert(0, _p)

B, N_ATOM, N_TOK = 4, 8192, 1024
ATOM_S, TFMR_S = 128, 384
LN_EPS = 1e-5
PAD_VAL = -1e9
P = 128
N_CORES = 8
TOK_SH = N_TOK // 2         # tokens per core (512)
N_WIN = TOK_SH // P         # 4 windows of 128 tokens
GH_W = ATOM_S + 33 + 35     # gather row: [a2q(128) | qr_a(33) | ha(35)] = 196
QN_W = ATOM_S + 33          # qn' row: [qn(128) | qr(33)] = 161
MASK_SENTINEL = 2048.0      # exact in f16, outside [0, 512)

LAST_RESULT = None


def _build(NT, gsched, ssched, sfirst, slast):
    from concourse import bacc, bass, mybir
    import concourse.tile as tile

    f32 = mybir.dt.float32
    f16 = mybir.dt.float16
    i32 = mybir.dt.int32
    Alu = mybir.AluOpType
    A_PAD = NT * P

    nc = bacc.Bacc(None, target_bir_lowering=False)

    q_t = nc.dram_tensor("q_t", [P, NT * QN_W], f16, kind="ExternalInput")
    qT = nc.dram_tensor("qT", [P, A_PAD], f16, kind="ExternalInput")
    aT = nc.dram_tensor("aT", [P, 3 * TOK_SH], f16, kind="ExternalInput")
    wA = nc.dram_tensor("wA", [P, 3 * GH_W], f16, kind="ExternalInput")
    wH = nc.dram_tensor("wH", [P, 68], f16, kind="ExternalInput")
    consts = nc.dram_tensor("consts", [P, GH_W + 6], f32, kind="ExternalInput")
    idxg = nc.dram_tensor("idxg", [1, A_PAD], f16, kind="ExternalInput")
    idxm_f = nc.dram_tensor("idxm_f", [P, NT], f16, kind="ExternalInput")

    at_out = nc.dram_tensor("at_out", [A_PAD, ATOM_S], f32, kind="ExternalOutput")
    r_out = nc.dram_tensor("r_out", [A_PAD, 3], f32, kind="ExternalOutput")
    res_out = nc.dram_tensor("res_out", [33, TOK_SH], f32, kind="ExternalOutput")

    with tile.TileContext(nc) as tc:
        with (
            tc.tile_pool(name="singles", bufs=1) as S,
            tc.tile_pool(name="work", bufs=4) as WK,
            tc.tile_pool(name="selp", bufs=6) as SELP,
            tc.tile_pool(name="mm_psum", bufs=4, space="PSUM") as MMP,
            tc.tile_pool(name="res_psum", bufs=4, space="PSUM") as RSP,
        ):
            # ---- persistent tiles ----
            q_buf = S.tile([P, NT * QN_W], f16)
            qT_buf = S.tile([P, A_PAD], f16)
            aT_sb = S.tile([P, 3 * TOK_SH], f16)
            wA_sb = S.tile([P, 3 * GH_W], f16)
            wH_sb = S.tile([P, 68], f16)
            consts_sb = S.tile([P, GH_W + 6], f32)
            idxm_sb = S.tile([P, NT], f16)
            idxT_sb = S.tile([P, A_PAD], f16)
            iota_pi = S.tile([P, N_WIN], i32)
            iota_pf = S.tile([P, N_WIN], f16)
            iotaT_i = S.tile([P, TOK_SH], i32)
            iotaT_f = S.tile([P, TOK_SH], f16)
            gh_sb = S.tile([P, N_WIN * GH_W], f16)
            Y_all = S.tile([P, 3 * NT], f32)
            stats_all = S.tile([P, 6 * NT], f32)
            rstd_sb = S.tile([P, NT], f32)
            z_sb = S.tile([P, NT], f32)
            rY_sb = S.tile([P, 3 * NT], f32)
            r_sb = S.tile([P, 3 * NT], f32)
            res_sb = S.tile([P, TOK_SH], f32)
            eps_sb = S.tile([P, 1], f32)

            # ---- loads & constants ----
            nq = NT * QN_W
            for c_ in range(4):
                lo_ = c_ * (nq // 4)
                hi_ = (c_ + 1) * (nq // 4) if c_ < 3 else nq
                nc.sync.dma_start(out=q_buf[:, lo_:hi_], in_=q_t[:, lo_:hi_])
            for c_ in range(2):
                sl = slice(c_ * (A_PAD // 2), (c_ + 1) * (A_PAD // 2))
                nc.sync.dma_start(out=qT_buf[:, sl], in_=qT[:, sl])
            nc.sync.dma_start(out=aT_sb[:], in_=aT[:])
            nc.sync.dma_start(out=wA_sb[:], in_=wA[:])
            nc.sync.dma_start(out=wH_sb[:], in_=wH[:])
            nc.sync.dma_start(out=consts_sb[:], in_=consts[:])
            nc.sync.dma_start(out=idxm_sb[:], in_=idxm_f[:])
            idxg_bcast = bass.AP(
                tensor=idxg[:].tensor, offset=0, ap=[[0, P], [1, A_PAD]]
            )
            nc.sync.dma_start(out=idxT_sb[:], in_=idxg_bcast)

            nc.vector.memset(eps_sb[:], LN_EPS)
            nc.gpsimd.iota(iota_pi[:], pattern=[[P, N_WIN]], base=0, channel_multiplier=1)
            nc.vector.tensor_copy(out=iota_pf[:], in_=iota_pi[:])
            nc.gpsimd.iota(iotaT_i[:], pattern=[[1, TOK_SH]], base=0, channel_multiplier=0)
            nc.vector.tensor_copy(out=iotaT_f[:], in_=iotaT_i[:])

            # ---- phase A: gh = [a2q | a2q@wres.T | a2q@[watom.T|Wg] + b_atom] ----
            for w in range(N_WIN):
                aps = MMP.tile([P, GH_W], f32, tag="mm", name=f"aps{w}")
                for c_ in range(3):
                    nc.tensor.matmul(
                        out=aps[:],
                        lhsT=aT_sb[:, c_ * TOK_SH + w * P : c_ * TOK_SH + (w + 1) * P],
                        rhs=wA_sb[:, c_ * GH_W : (c_ + 1) * GH_W],
                        start=(c_ == 0),
                        stop=(c_ == 2),
                    )
                nc.vector.tensor_tensor(
                    out=gh_sb[:, w * GH_W : (w + 1) * GH_W],
                    in0=aps[:],
                    in1=consts_sb[:, 0:GH_W],
                    op=Alu.add,
                )

            # ---- main loop over atom tiles ----
            res_tiles = {}
            for t in range(NT):
                tsl = slice(t * P, (t + 1) * P)
                gps = MMP.tile([P, GH_W], f32, tag="mm", name=f"gps{t}")
                gws = gsched[t]
                for j, w in enumerate(gws):
                    selT = SELP.tile([P, P], f16, tag="selT", name=f"selT{t}_{w}")
                    nc.vector.tensor_tensor(
                        out=selT[:],
                        in0=idxT_sb[:, tsl],
                        in1=iota_pf[:, w : w + 1].to_broadcast([P, P]),
                        op=Alu.is_equal,
                    )
                    nc.tensor.matmul(
                        out=gps[:],
                        lhsT=selT[:],
                        rhs=gh_sb[:, w * GH_W : (w + 1) * GH_W],
                        start=(j == 0),
                        stop=False,
                        skip_group_check=True,
                    )
                # head contribution accumulates onto gps[:, 128:196]
                nc.tensor.matmul(
                    out=gps[:, ATOM_S:GH_W],
                    lhsT=qT_buf[:, tsl],
                    rhs=wH_sb[:],
                    start=False,
                    stop=True,
                    skip_group_check=True,
                )

                # qn' = [q|0] + gps[:, 0:161]   (cols 0:128 = qn, 128:161 = qr)
                qn = WK.tile([P, QN_W], f16, tag="qn", name=f"qn{t}")
                nc.vector.tensor_tensor(
                    out=qn[:],
                    in0=q_buf[:, t * QN_W : (t + 1) * QN_W],
                    in1=gps[:, 0:QN_W],
                    op=Alu.add,
                )

                # LN stats (count, mean, count*var) straight into column buffer
                nc.vector.bn_stats(
                    out=stats_all[:, 6 * t : 6 * t + 6], in_=qn[:, 0:ATOM_S]
                )

                # atom_type tile: -1e9 fill + strided copy of (small + b_atom)
                at = WK.tile([P, ATOM_S], f32, tag="at", name=f"at{t}")
                nc.gpsimd.memset(at[:], PAD_VAL)
                at4 = at[:].rearrange("p (k f) -> p k f", f=4)
                gps3 = gps[:].rearrange("p (k f) -> p k f", f=1)
                nc.scalar.copy(out=at4[:, :, 0:1], in_=gps3[:, 161:193, :])
                nc.sync.dma_start(out=at_out[tsl, :], in_=at[:])

                # Y staging for r_update
                nc.scalar.copy(
                    out=Y_all[:, 3 * t : 3 * t + 3], in_=gps[:, 193:196]
                )

                # segment-sum: res_partialT[33, tok] += qr.T @ sel
                for w in ssched[t]:
                    sel = SELP.tile([P, P], f16, tag="sel", name=f"sel{t}_{w}")
                    nc.vector.tensor_tensor(
                        out=sel[:],
                        in0=idxm_sb[:, t : t + 1].to_broadcast([P, P]),
                        in1=iotaT_f[:, w * P : (w + 1) * P],
                        op=Alu.is_equal,
                    )
                    if w not in res_tiles:
                        res_tiles[w] = RSP.tile(
                            [33, P], f32, tag="resT", name=f"resT{w}"
                        )
                    nc.tensor.matmul(
                        out=res_tiles[w][:],
                        lhsT=qn[:, ATOM_S:QN_W],
                        rhs=sel[:],
                        start=(t == sfirst[w]),
                        stop=(t == slast[w]),
                        skip_group_check=True,
                    )
                    if t == slast[w]:
                        nc.scalar.copy(
                            out=res_sb[0:33, w * P : (w + 1) * P],
                            in_=res_tiles[w][:],
                        )
                        del res_tiles[w]

            # unhit windows (rare): zero their res columns
            for w in range(N_WIN):
                if w not in sfirst:
                    nc.vector.memset(res_sb[0:33, w * P : (w + 1) * P], 0.0)
            nc.sync.dma_start(out=res_out[:], in_=res_sb[0:33, :])

            # ---- batched r_update epilogue ----
            # bn_stats emits two (count=64, mean, 64*var) triplets per tile:
            #   mu  = (m0 + m1)/2           (the /2 is folded into -sWg/2, bPos ok)
            #   var = (cv0 + cv1)/128 + (m0 - m1)^2/4
            st6 = stats_all[:].rearrange("p (t k) -> p t k", k=6)
            m0, m1 = st6[:, :, 1:2], st6[:, :, 4:5]
            cv0, cv1 = st6[:, :, 2:3], st6[:, :, 5:6]
            msum3 = z_sb[:].rearrange("p (t one) -> p t one", one=1)  # reuse z_sb
            nc.vector.tensor_tensor(out=msum3, in0=m0, in1=m1, op=Alu.add)
            vtmp = S.tile([P, NT], f32)
            dmt = S.tile([P, NT], f32)
            vtmp3 = vtmp[:].rearrange("p (t one) -> p t one", one=1)
            dmt3 = dmt[:].rearrange("p (t one) -> p t one", one=1)
            nc.vector.tensor_tensor(out=vtmp3, in0=cv0, in1=cv1, op=Alu.add)
            nc.vector.tensor_tensor(out=dmt3, in0=m0, in1=m1, op=Alu.subtract)
            nc.vector.tensor_tensor(out=dmt[:], in0=dmt[:], in1=dmt[:], op=Alu.mult)
            nc.vector.tensor_scalar(
                out=vtmp[:], in0=vtmp[:], scalar1=1.0 / ATOM_S, scalar2=None,
                op0=Alu.mult,
            )
            nc.vector.tensor_scalar(
                out=dmt[:], in0=dmt[:], scalar1=0.25, scalar2=None, op0=Alu.mult
            )
            nc.vector.tensor_tensor(out=vtmp[:], in0=vtmp[:], in1=dmt[:], op=Alu.add)
            rstd3 = rstd_sb[:].rearrange("p (t one) -> p t one", one=1)
            nc.scalar.activation(
                out=rstd_sb[:],
                in_=vtmp[:],
                func=mybir.ActivationFunctionType.Sqrt,
                bias=eps_sb[:],
                scale=1.0,
            )
            nc.vector.reciprocal(out=rstd_sb[:], in_=rstd_sb[:])
            # z = (m0 + m1) * rstd   (true mu*rstd*2; halved via -sWg/2 const)
            z3 = msum3
            nc.vector.tensor_tensor(out=z_sb[:], in0=z_sb[:], in1=rstd_sb[:], op=Alu.mult)
            Y3 = Y_all[:].rearrange("p (t o) -> p t o", o=3)
            rY3 = rY_sb[:].rearrange("p (t o) -> p t o", o=3)
            nc.vector.tensor_tensor(
                out=rY3, in0=Y3, in1=rstd3.to_broadcast([P, NT, 3]), op=Alu.mult
            )
            negsWg3 = (
                consts_sb[:, GH_W : GH_W + 3]
                .rearrange("p (one o) -> p one o", one=1)
                .to_broadcast([P, NT, 3])
            )
            bpos3 = (
                consts_sb[:, GH_W + 3 : GH_W + 6]
                .rearrange("p (one o) -> p one o", one=1)
                .to_broadcast([P, NT, 3])
            )
            r3 = r_sb[:].rearrange("p (t o) -> p t o", o=3)
            nc.vector.tensor_tensor(
                out=r3, in0=z3.to_broadcast([P, NT, 3]), in1=negsWg3, op=Alu.mult
            )
            nc.vector.tensor_tensor(out=r3, in0=r3, in1=rY3, op=Alu.add)
            nc.vector.tensor_tensor(out=r3, in0=r3, in1=bpos3, op=Alu.add)
            nc.sync.dma_start(
                out=r_out[:].rearrange("(t p) o -> p t o", p=P), in_=r3
            )

    nc.compile()
    return nc


def kernel(
    a,
    q,
    c,
    atom_tok_idx,
    atom_to_token,
    atom_pad_mask,
    W_a2q,
    ln_gamma,
    ln_beta,
    W_pos,
    W_res,
    b_res,
    W_atom,
    b_atom,
    allowed_idx,
    **_unused,
):
    global LAST_RESULT
    from concourse.bass_utils import run_bass_kernel_spmd

    f16 = np.float16
    a = np.asarray(a, np.float32)
    q = np.asarray(q, np.float32)
    idx_all = np.asarray(atom_tok_idx).astype(np.int64)
    mask_all = np.asarray(atom_pad_mask).astype(bool)
    W_a2q = np.asarray(W_a2q, np.float32)
    ln_gamma = np.asarray(ln_gamma, np.float32)
    ln_beta = np.asarray(ln_beta, np.float32)
    W_pos = np.asarray(W_pos, np.float32)
    W_res = np.asarray(W_res, np.float32)
    b_res = np.asarray(b_res, np.float32)
    W_atom = np.asarray(W_atom, np.float32)
    b_atom = np.asarray(b_atom, np.float32)

    # ---- shard boundaries (token-sharded halves) ----
    cores = []
    for core in range(N_CORES):
        b, h = divmod(core, 2)
        cut = int(np.searchsorted(idx_all[b], TOK_SH))
        lo, hi = (0, cut) if h == 0 else (cut, N_ATOM)
        cores.append({"b": b, "h": h, "lo": lo, "hi": hi, "cnt": hi - lo})
    max_cnt = max(ci["cnt"] for ci in cores)
    NT = max(1, (max_cnt + P - 1) // P)
    A_PAD = NT * P

    # ---- per-core indices and union schedules ----
    gsched_sets = [set() for _ in range(NT)]
    ssched_sets = [set() for _ in range(NT)]
    for ci in cores:
        b, h, lo, hi, cnt = ci["b"], ci["h"], ci["lo"], ci["hi"], ci["cnt"]
        idx_reb = (idx_all[b, lo:hi] - h * TOK_SH).astype(np.float32)
        mask = mask_all[b, lo:hi]
        idxg = np.full(A_PAD, MASK_SENTINEL, np.float32)
        idxg[:cnt] = idx_reb
        idxm = np.full(A_PAD, MASK_SENTINEL, np.float32)
        idxm[:cnt] = np.where(mask, idx_reb, MASK_SENTINEL)
        ci["idxg"] = idxg
        ci["idxm"] = idxm
        for t in range(NT):
            for w in np.unique(idxg[t * P : (t + 1) * P] // P):
                if 0 <= w < N_WIN:
                    gsched_sets[t].add(int(w))
            for w in np.unique(idxm[t * P : (t + 1) * P] // P):
                if 0 <= w < N_WIN:
                    ssched_sets[t].add(int(w))
    gsched = [sorted(s) if s else [0] for s in gsched_sets]
    ssched = [sorted(s) if s else [0] for s in ssched_sets]
    sfirst, slast = {}, {}
    for t in range(NT):
        for w in ssched[t]:
            sfirst.setdefault(w, t)
            slast[w] = t

    # ---- host-folded weights ----
    Wg = ln_gamma[:, None] * W_pos.T                     # [128, 3]
    sWg = Wg.sum(axis=0)
    bpos = ln_beta @ W_pos.T
    wH_f = np.hstack([W_res.T, W_atom.T, Wg]).astype(np.float32)   # [128, 68]
    wA_f = np.hstack([np.eye(ATOM_S, dtype=np.float32), wH_f])     # [128, 196]
    wA_full = W_a2q.T @ wA_f                                       # [384, 196]
    wA_np = (
        wA_full.reshape(3, P, GH_W).transpose(1, 0, 2).reshape(P, 3 * GH_W).astype(f16)
    )
    wH_np = wH_f.astype(f16)
    cvec = np.concatenate(
        [
            np.zeros(ATOM_S + 33, np.float32),
            b_atom,
            np.zeros(3, np.float32),
            -sWg / 2.0,
            bpos,
        ]
    ).astype(np.float32)
    consts_np = np.broadcast_to(cvec[None, :], (P, GH_W + 6)).copy()

    # ---- per-core input maps ----
    in_maps = []
    for ci in cores:
        b, h, lo, hi, cnt = ci["b"], ci["h"], ci["lo"], ci["hi"], ci["cnt"]
        q_sh = np.zeros((A_PAD, ATOM_S), np.float32)
        q_sh[:cnt] = q[b, lo:hi]
        qpad = np.zeros((A_PAD, QN_W), np.float32)
        qpad[:, :ATOM_S] = q_sh
        q_t_np = (
            qpad.reshape(NT, P, QN_W).transpose(1, 0, 2).reshape(P, NT * QN_W)
            .astype(f16)
        )
        qT_np = np.ascontiguousarray(q_sh.T).astype(f16)
        aT_np = (
            a[b].T[:, h * TOK_SH : (h + 1) * TOK_SH]
            .reshape(3, P, TOK_SH)
            .transpose(1, 0, 2)
            .reshape(P, 3 * TOK_SH)
            .astype(f16)
        )
        in_maps.append(
            {
                "q_t": q_t_np,
                "qT": qT_np,
                "aT": aT_np,
                "wA": wA_np,
                "wH": wH_np,
                "consts": consts_np,
                "idxg": ci["idxg"][None, :].astype(f16),
                "idxm_f": ci["idxm"].reshape(NT, P).T.astype(f16).copy(),
            }
        )

    nc = _build(NT, gsched, ssched, sfirst, slast)
    LAST_RESULT = run_bass_kernel_spmd(nc, in_maps, core_ids=list(range(N_CORES)))
    results = LAST_RESULT.results

    # ---- host assembly ----
    r_update = np.empty((B, N_ATOM, 3), np.float32)
    atom_type = np.empty((B, N_ATOM, ATOM_S), np.float32)
    res_type = np.broadcast_to(b_res.astype(np.float32), (B, N_TOK, 33)).copy()
    for core, ci in enumerate(cores):
        b, h, lo, hi, cnt = ci["b"], ci["h"], ci["lo"], ci["hi"], ci["cnt"]
        r_update[b, lo:hi] = results[core]["r_out"][:cnt]
        atom_type[b, lo:hi] = results[core]["at_out"][:cnt]
        res_type[b, h * TOK_SH : (h + 1) * TOK_SH] += results[core]["res_out"].T
    return (r_update, res_type, atom_type)
